# revision 46
# baseline (speedup 1.0000x reference)
"""GTN (graph transformer network) meta-path kernel for TRN2, 8 NeuronCores.

Math (reference):
    Ap = A transposed to [E, N, N]
    a  = sum_e softmax(w1_0)[c,e] * Ap[e]      (per channel c)
    b  = sum_e softmax(w2_0)[c,e] * Ap[e]
    H  = a @ b
    twice:  H = normalize(H) @ gtconv(Ap, w)   (normalize = zero diag, col-scale)
    out = symmetrized mean over channels.

Sharding: channel-parallel — core c computes channel c end to end (the four
softmax mixes differ only in the tiny [E] weight vector, fed per-core).  Each
core locally symmetrizes G_c = (H''_c + H''_c^T)/16, one ReduceScatter sums
the G_c and leaves each core a 256-row band of the result; the host stacks
the 8 bands.

All heavy compute runs in fp8 (e4m3):
 - A is uploaded pre-permuted fp8; one DoubleRow PE pass computes all four
   mixes straight into natural-layout DRAM (no packed/unpack round trip).
 - The three chained 2048^3 GEMMs run fp8 DoubleRow (2 k-tiles per pass).
   The normalized intermediates are scaled by S=1024 so their ~1/N entries
   sit near 0.5 where e4m3 has full precision; S cancels inside normalize
   and 1/(16*S) is folded into the GEMM3 eviction scale.

On-device formulation works with TRANSPOSED intermediates Ht = H^T so that
every GEMM's moving operand is the previous GEMM's output as-is, and
normalization becomes row sums (free-dim reduce) + per-partition scale.
"""

import numpy as np

N = 2048
E = 8
C = 8
P = 128
NCORES = 8
S_SCALE = 1024.0

_PROGRAM = None


def _softmax_rows(w: np.ndarray) -> np.ndarray:
    """w: [C, E, 1, 1] -> softmax over E, float64 precision, returns [C, E]."""
    x = w.reshape(C, E).astype(np.float64)
    x = x - x.max(axis=1, keepdims=True)
    ex = np.exp(x)
    return ex / ex.sum(axis=1, keepdims=True)


def _build_program():
    import concourse.bacc as bacc
    import concourse.mybir as mybir
    import concourse.tile as tile
    from concourse.masks import make_identity

    f32 = mybir.dt.float32
    bf16 = mybir.dt.bfloat16
    fp8 = mybir.dt.float8e4
    AX = mybir.AxisListType.X
    MUL = mybir.AluOpType.mult
    ADD = mybir.AluOpType.add
    NE = mybir.AluOpType.not_equal
    COPY = mybir.ActivationFunctionType.Copy
    RECIP = mybir.ActivationFunctionType.Reciprocal
    DR = mybir.MatmulPerfMode.DoubleRow

    nc = bacc.Bacc("TRN2")
    A3_ext = nc.dram_tensor("At3", [P, P, N], fp8, kind="ExternalInput")
    w4_ext = nc.dram_tensor("w4d", [P, 2, P], fp8, kind="ExternalInput")
    # bf16 band output; the host gather upcasts to f32
    out_ext = nc.dram_tensor(
        "out", [N // NCORES, N], bf16, kind="ExternalOutput"
    )

    with tile.TileContext(nc) as tc:
        with (
            tc.tile_pool(name="dram", bufs=1, space="DRAM") as dpool,
            tc.tile_pool(name="const", bufs=1) as cpool,
        ):
            # the four mixes (a, b, g1, g2) row-interleaved as [row, q, col]:
            # with mix psum rows ordered (h, x, q), a 64-partition stg slice
            # maps to one fully contiguous 128KB DRAM write
            nat_il = dpool.tile([N, 4, N], fp8, name="natil")
            nat = [nat_il[:, q, :] for q in range(4)]
            gsym = dpool.tile([N, N], bf16, name="gsym")
            g_sh = dpool.tile([N // NCORES, N], bf16, name="gsh")

            # --- constants ---
            w4_sb = cpool.tile([P, 2, P], fp8)
            nc.sync.dma_start(out=w4_sb[:], in_=w4_ext[:])
            ident8 = cpool.tile([P, P], fp8)
            make_identity(nc, ident8[:])
            identb = cpool.tile([P, P], bf16)
            make_identity(nc, identb[:])
            # diag masks: masks[:, v, y] = 0 where y == p + v*128 else 1
            masks = cpool.tile([P, 4, 512], f32)
            nc.gpsimd.memset(masks[:], 1.0)
            for v in range(4):
                nc.gpsimd.affine_select(
                    out=masks[:, v],
                    in_=masks[:, v],
                    compare_op=NE,
                    fill=0.0,
                    base=v * P,
                    pattern=[[-1, 512]],
                    channel_multiplier=1,
                )

            # ======== Phase 1: all four mixes, one DoubleRow PE pass ========
            # a3t partitions hold (k16, e); the duplicated block-diag weight
            # computes two row blocks (h) of all four mixes per matmul:
            # pm row (h, q, x) = mix_q[16*(4*ld4 + 2*half + h) + x, :].
            with (
                tc.tile_pool(name="mix", bufs=4) as mpool,
                tc.tile_pool(name="mixst", bufs=3) as spool,
                tc.tile_pool(name="mixps", bufs=6, space="PSUM") as mpsum,
            ):
                # software-pipelined: loads prefetched 2 iterations ahead of
                # the compute/writes so the sync ring's loads never sit
                # behind writes that wait on this iteration's copies
                PF = 2
                a3ts = {}
                for it in range(32 + PF):
                    if it < 32:
                        a3t = mpool.tile([P, 4, N], fp8, tag="a3t")
                        nc.sync.dma_start(
                            out=a3t[:],
                            in_=A3_ext[4 * it : 4 * it + 4].rearrange(
                                "b p j -> p b j"
                            ),
                        )
                        a3ts[it] = a3t
                    if it < PF:
                        continue
                    ld4 = it - PF
                    a3t = a3ts.pop(ld4)
                    stg = spool.tile([P, 2, N], fp8, tag="stg")
                    for half in range(2):
                        for jc in range(4):
                            pm = mpsum.tile([P, 512], f32, tag="pm")
                            nc.tensor.matmul(
                                pm[:],
                                lhsT=w4_sb[:],
                                rhs=a3t[
                                    :,
                                    2 * half : 2 * half + 2,
                                    jc * 512 : (jc + 1) * 512,
                                ],
                                start=True,
                                stop=True,
                                perf_mode=DR,
                            )
                            dst = stg[:, half, jc * 512 : (jc + 1) * 512]
                            if (half * 4 + jc) % 4 == 3:
                                nc.scalar.copy(dst, pm[:])
                            else:
                                nc.vector.tensor_copy(out=dst, in_=pm[:])
                    # natural-layout row band: rows 64*ld4 .. 64*ld4+64 of
                    # each mix; one DMA per (half, h): 64 source partitions,
                    # one contiguous 128KB destination block
                    wengs = [nc.scalar, nc.gpsimd, nc.gpsimd, nc.scalar]
                    for half in range(2):
                        for h in range(2):
                            base = 64 * ld4 + 32 * half + 16 * h
                            wengs[2 * half + h].dma_start(
                                out=nat_il[base : base + 16].rearrange(
                                    "x q j -> (x q) j"
                                ),
                                in_=stg[h * 64 : h * 64 + 64, half, :],
                            )

            # =========== Phases 2-4: three chained GEMMs (fp8 DR) ===========
            with (
                tc.tile_pool(name="big", bufs=1) as bigpool,
                tc.tile_pool(name="gw", bufs=3) as gpool,
                tc.tile_pool(name="nrm", bufs=4) as npool,
            ):
                mv = [
                    bigpool.tile([P, 16, N], fp8, tag="mv0", name="mva"),
                    bigpool.tile([P, 16, N], fp8, tag="mv1", name="mvb"),
                ]
                h2t_sb = bigpool.tile([P, 16, N], bf16, tag="h2t", name="h2t")

                # Build mv0 = a^T chunks by PE-transposing nat0 blocks (fp8)
                anat_v = nat[0].rearrange("(ib p) k -> p ib k", p=P)
                with tc.tile_pool(name="tps", bufs=2, space="PSUM") as tpsum:
                    for kc in range(16):
                        ld = gpool.tile([P, 16, P], fp8, tag="ld")
                        # strided 128B-segment read: split across both HWDGE
                        # rings for descriptor-rate headroom
                        nc.sync.dma_start(
                            out=ld[:, 0:8, :],
                            in_=anat_v[:, 0:8, kc * P : (kc + 1) * P],
                        )
                        nc.scalar.dma_start(
                            out=ld[:, 8:16, :],
                            in_=anat_v[:, 8:16, kc * P : (kc + 1) * P],
                        )
                        for ib4 in range(4):
                            # fp8 transpose writes PSUM at element step 2
                            tp = tpsum.tile(
                                [P, 512, 2], fp8, tag=f"tp{ib4 % 2}", name="tp"
                            )
                            for g in range(4):
                                nc.tensor.transpose(
                                    tp[:, g * P : (g + 1) * P, 0],
                                    ld[:, ib4 * 4 + g, :],
                                    ident8[:],
                                )
                            if ib4 % 2 == 0:
                                nc.vector.tensor_copy(
                                    out=mv[0][
                                        :, kc, ib4 * 512 : (ib4 + 1) * 512
                                    ],
                                    in_=tp[:, :, 0],
                                )
                            else:
                                nc.scalar.copy(
                                    mv[0][:, kc, ib4 * 512 : (ib4 + 1) * 512],
                                    tp[:, :, 0],
                                )

                def gemm(qi, rhs_res, out_res, gpsum):
                    """Transposed-chain GEMM: out = mix_q^T @ rhs, fp8 DR.

                    qi: q index in nat (1=b, 2=g1, 3=g2).
                    rhs_res: SBUF-resident moving operand [P, 16, N] fp8,
                        holding S * (previous normalized intermediate)^T.
                    out_res: SBUF [P, 16, N] fp8 -> normalize, evict with
                        dinv*S; None -> evict bf16 to h2t_sb with 1/(16*S).
                    """
                    natv = nat[qi].rearrange("(kc p) j -> p kc j", p=P)
                    for ms in range(16):
                        bts = gpool.tile([P, 16, P], fp8, tag="bts")
                        # split the strided stationary load across both rings
                        nc.sync.dma_start(
                            out=bts[:, 0:8, :],
                            in_=natv[:, 0:8, ms * P : (ms + 1) * P],
                        )
                        nc.scalar.dma_start(
                            out=bts[:, 8:16, :],
                            in_=natv[:, 8:16, ms * P : (ms + 1) * P],
                        )
                        ps = [
                            gpsum.tile(
                                [P, 512], f32, tag=f"ps{ic}", name=f"ps{ic}"
                            )
                            for ic in range(4)
                        ]
                        for kc2 in range(8):
                            for ic in range(4):
                                nc.tensor.matmul(
                                    ps[ic][:],
                                    lhsT=bts[:, 2 * kc2 : 2 * kc2 + 2, :],
                                    rhs=rhs_res[
                                        :,
                                        2 * kc2 : 2 * kc2 + 2,
                                        ic * 512 : (ic + 1) * 512,
                                    ],
                                    start=(kc2 == 0),
                                    stop=(kc2 == 7),
                                    perf_mode=DR,
                                )
                        if out_res is not None:
                            dc = (ms * P) // 512
                            v = ms % 4
                            degp = npool.tile([P, 4], f32, tag="degp")
                            # zero the diagonal in place + row-sum of masked
                            nc.vector.scalar_tensor_tensor(
                                out=ps[dc][:],
                                in0=ps[dc][:],
                                scalar=1.0,
                                in1=masks[:, v],
                                op0=MUL,
                                op1=MUL,
                                accum_out=degp[:, dc : dc + 1],
                            )
                            for ic in range(4):
                                if ic != dc:
                                    nc.vector.tensor_reduce(
                                        degp[:, ic : ic + 1], ps[ic][:], AX, ADD
                                    )
                            degs = npool.tile([P, 1], f32, tag="degs")
                            nc.vector.tensor_reduce(degs[:], degp[:], AX, ADD)
                            # dinv = S/deg, so the fp8 store sits near 0.5
                            degss = npool.tile([P, 1], f32, tag="degss")
                            nc.vector.tensor_scalar_mul(
                                degss[:], degs[:], 1.0 / S_SCALE
                            )
                            dinv = npool.tile([P, 1], f32, tag="dinv")
                            nc.vector.reciprocal(dinv[:], degss[:])
                            for ic in range(4):
                                nc.scalar.activation(
                                    out_res[:, ms, ic * 512 : (ic + 1) * 512],
                                    ps[ic][:],
                                    COPY,
                                    scale=dinv[:],
                                )
                        else:
                            for ic in range(4):
                                nc.scalar.activation(
                                    h2t_sb[:, ms, ic * 512 : (ic + 1) * 512],
                                    ps[ic][:],
                                    COPY,
                                    scale=1.0 / (16.0 * S_SCALE),
                                )

                with tc.tile_pool(name="gps", bufs=2, space="PSUM") as gpsum:
                    # GEMM1: Ht = b^T a^T ; normalize -> S*Hnt in mv[1]
                    gemm(1, mv[0], mv[1], gpsum)
                    # GEMM2: H't = g1^T (S Hnt) ; normalize -> S*H'nt
                    mv0b = bigpool.tile([P, 16, N], fp8, tag="mv0")
                    gemm(2, mv[1], mv0b, gpsum)
                    # GEMM3: H''t/16 = g2^T (S H'nt)/(16 S) -> h2t_sb (bf16)
                    gemm(3, mv0b, None, gpsum)

                # ===== Phase 5: local symmetrize G = (H'' + H''^T)/16 =====
                with tc.tile_pool(name="sps", bufs=2, space="PSUM") as spsum:
                    for ms in range(16):
                        tps = [
                            spsum.tile(
                                [P, 512], bf16, tag=f"sp{b4}", name="tps"
                            )
                            for b4 in range(4)
                        ]
                        for kc in range(16):
                            nc.tensor.transpose(
                                tps[kc // 4][
                                    :, (kc % 4) * P : (kc % 4 + 1) * P
                                ],
                                h2t_sb[:, kc, ms * P : (ms + 1) * P],
                                identb[:],
                            )
                        gst = gpool.tile([P, N], bf16, tag="gst")
                        for b4 in range(4):
                            eng = nc.vector
                            eng.scalar_tensor_tensor(
                                out=gst[:, b4 * 512 : (b4 + 1) * 512],
                                in0=h2t_sb[:, ms, b4 * 512 : (b4 + 1) * 512],
                                scalar=1.0,
                                in1=tps[b4][:],
                                op0=MUL,
                                op1=ADD,
                            )
                        weng = nc.scalar if ms % 2 else nc.sync
                        weng.dma_start(
                            out=gsym[ms * P : (ms + 1) * P, :], in_=gst[:]
                        )

                # ===== Phase 6: ReduceScatter -> this core's 256-row band ===
                nc.gpsimd.collective_compute(
                    "ReduceScatter",
                    ADD,
                    replica_groups=[list(range(NCORES))],
                    ins=[gsym[:].opt()],
                    outs=[g_sh[:].opt()],
                )

                # ===== Phase 7: copy the band out (d2d; host upcasts) =====
                nc.sync.dma_start(out=out_ext[:], in_=g_sh[:])

    nc.compile()
    return nc


def _get_program():
    global _PROGRAM
    if _PROGRAM is None:
        _PROGRAM = _build_program()
    return _PROGRAM


def _make_w4d(sws) -> np.ndarray:
    """Duplicated block-diagonal mix weights [128, 2, 128].

    w4d[(x*8+e), h, h2*64 + x*4 + q] = sws[q][e] if h==h2.
    With DoubleRow the two k-tiles (h) of the moving operand produce the
    two row blocks h2 of the packed (x, q) mix outputs; q is innermost so
    the mix eviction hits the row-interleaved nat layout contiguously.
    """
    w4d = np.zeros((P, 2, P), np.float32)
    for q, sw in enumerate(sws):
        for x in range(16):
            for h in range(2):
                w4d[x * 8 : (x + 1) * 8, h, h * 64 + x * 4 + q] = sw.astype(
                    np.float32
                )
    return w4d


def _prep_inputs(A, w1_0, w2_0, w_1, w_2):
    import ml_dtypes

    e4 = ml_dtypes.float8_e4m3
    swa = _softmax_rows(np.asarray(w1_0))
    swb = _softmax_rows(np.asarray(w2_0))
    sg1 = _softmax_rows(np.asarray(w_1))
    sg2 = _softmax_rows(np.asarray(w_2))

    a8 = np.asarray(A, dtype=np.float32)[0].astype(e4)  # [k, j, e]
    # At3[kb, (k16 e), j] = A[16*kb + k16, j, e]
    at3 = np.ascontiguousarray(a8.transpose(0, 2, 1).reshape(P, P, N))
    in_maps = []
    for c in range(NCORES):
        w4d = _make_w4d([swa[c], swb[c], sg1[c], sg2[c]]).astype(e4)
        in_maps.append({"At3": at3, "w4d": w4d})
    return in_maps


def kernel(A, w1_0, w2_0, w_1, w_2):
    from concourse.bass_utils import run_bass_kernel_spmd

    in_maps = _prep_inputs(A, w1_0, w2_0, w_1, w_2)
    nc = _get_program()
    res = run_bass_kernel_spmd(nc, in_maps, list(range(NCORES)))
    return np.concatenate(
        [
            np.asarray(res.results[k]["out"]).astype(np.float32)
            for k in range(NCORES)
        ],
        axis=0,
    )


# revision 47
# speedup vs baseline: 1.0452x; 1.0452x over previous
"""GTN (graph transformer network) meta-path kernel for TRN2, 8 NeuronCores.

Math (reference):
    Ap = A transposed to [E, N, N]
    a  = sum_e softmax(w1_0)[c,e] * Ap[e]      (per channel c)
    b  = sum_e softmax(w2_0)[c,e] * Ap[e]
    H  = a @ b
    twice:  H = normalize(H) @ gtconv(Ap, w)   (normalize = zero diag, col-scale)
    out = symmetrized mean over channels.

Sharding: channel-parallel — core c computes channel c end to end (the four
softmax mixes differ only in the tiny [E] weight vector, fed per-core).  Each
core locally symmetrizes G_c = (H''_c + H''_c^T)/16, one ReduceScatter sums
the G_c and leaves each core a 256-row band of the result; the host stacks
the 8 bands.

All heavy compute runs in fp8 (e4m3):
 - A is uploaded pre-permuted fp8; one DoubleRow PE pass computes all four
   mixes straight into natural-layout DRAM (no packed/unpack round trip).
 - The three chained 2048^3 GEMMs run fp8 DoubleRow (2 k-tiles per pass).
   The normalized intermediates are scaled by S=1024 so their ~1/N entries
   sit near 0.5 where e4m3 has full precision; S cancels inside normalize
   and 1/(16*S) is folded into the GEMM3 eviction scale.

On-device formulation works with TRANSPOSED intermediates Ht = H^T so that
every GEMM's moving operand is the previous GEMM's output as-is, and
normalization becomes row sums (free-dim reduce) + per-partition scale.
"""

import numpy as np

N = 2048
E = 8
C = 8
P = 128
NCORES = 8
S_SCALE = 1024.0

_PROGRAM = None


def _softmax_rows(w: np.ndarray) -> np.ndarray:
    """w: [C, E, 1, 1] -> softmax over E, float64 precision, returns [C, E]."""
    x = w.reshape(C, E).astype(np.float64)
    x = x - x.max(axis=1, keepdims=True)
    ex = np.exp(x)
    return ex / ex.sum(axis=1, keepdims=True)


def _build_program():
    import concourse.bacc as bacc
    import concourse.mybir as mybir
    import concourse.tile as tile
    from concourse.masks import make_identity

    f32 = mybir.dt.float32
    bf16 = mybir.dt.bfloat16
    fp8 = mybir.dt.float8e4
    AX = mybir.AxisListType.X
    MUL = mybir.AluOpType.mult
    ADD = mybir.AluOpType.add
    NE = mybir.AluOpType.not_equal
    COPY = mybir.ActivationFunctionType.Copy
    RECIP = mybir.ActivationFunctionType.Reciprocal
    DR = mybir.MatmulPerfMode.DoubleRow

    nc = bacc.Bacc("TRN2")
    A3_ext = nc.dram_tensor("At3", [P, P, N], fp8, kind="ExternalInput")
    w4_ext = nc.dram_tensor("w4d", [P, 2, P], fp8, kind="ExternalInput")
    # bf16 band output; the host gather upcasts to f32
    out_ext = nc.dram_tensor(
        "out", [N // NCORES, N], bf16, kind="ExternalOutput"
    )

    with tile.TileContext(nc) as tc:
        with (
            tc.tile_pool(name="dram", bufs=1, space="DRAM") as dpool,
            tc.tile_pool(name="const", bufs=1) as cpool,
        ):
            # the four mixes (a, b, g1, g2) row-interleaved as [row, q, col]:
            # with mix psum rows ordered (h, x, q), a 64-partition stg slice
            # maps to one fully contiguous 128KB DRAM write
            nat_il = dpool.tile([N, 4, N], fp8, name="natil")
            nat = [nat_il[:, q, :] for q in range(4)]
            gsym = dpool.tile([N, N], bf16, name="gsym")
            g_sh = dpool.tile([N // NCORES, N], bf16, name="gsh")

            # --- constants ---
            w4_sb = cpool.tile([P, 2, P], fp8)
            nc.sync.dma_start(out=w4_sb[:], in_=w4_ext[:])
            ident8 = cpool.tile([P, P], fp8)
            make_identity(nc, ident8[:])
            identb = cpool.tile([P, P], bf16)
            make_identity(nc, identb[:])
            # diag masks: masks[:, v, y] = 0 where y == p + v*128 else 1
            masks = cpool.tile([P, 4, 512], f32)
            nc.gpsimd.memset(masks[:], 1.0)
            for v in range(4):
                nc.gpsimd.affine_select(
                    out=masks[:, v],
                    in_=masks[:, v],
                    compare_op=NE,
                    fill=0.0,
                    base=v * P,
                    pattern=[[-1, 512]],
                    channel_multiplier=1,
                )

            # ======== Phase 1: all four mixes, one DoubleRow PE pass ========
            # a3t partitions hold (k16, e); the duplicated block-diag weight
            # computes two row blocks (h) of all four mixes per matmul:
            # pm row (h, q, x) = mix_q[16*(4*ld4 + 2*half + h) + x, :].
            with (
                tc.tile_pool(name="mix", bufs=4) as mpool,
                tc.tile_pool(name="mixst", bufs=3) as spool,
                tc.tile_pool(name="mixps", bufs=6, space="PSUM") as mpsum,
            ):
                # software-pipelined: loads prefetched 2 iterations ahead of
                # the compute/writes so the sync ring's loads never sit
                # behind writes that wait on this iteration's copies
                PF = 2
                a3ts = {}
                for it in range(32 + PF):
                    if it < 32:
                        a3t = mpool.tile([P, 4, N], fp8, tag="a3t")
                        nc.sync.dma_start(
                            out=a3t[:],
                            in_=A3_ext[4 * it : 4 * it + 4].rearrange(
                                "b p j -> p b j"
                            ),
                        )
                        a3ts[it] = a3t
                    if it < PF:
                        continue
                    ld4 = it - PF
                    a3t = a3ts.pop(ld4)
                    stg = spool.tile([P, 2, N], fp8, tag="stg")
                    for half in range(2):
                        for jc in range(4):
                            pm = mpsum.tile([P, 512], f32, tag="pm")
                            nc.tensor.matmul(
                                pm[:],
                                lhsT=w4_sb[:],
                                rhs=a3t[
                                    :,
                                    2 * half : 2 * half + 2,
                                    jc * 512 : (jc + 1) * 512,
                                ],
                                start=True,
                                stop=True,
                                perf_mode=DR,
                            )
                            dst = stg[:, half, jc * 512 : (jc + 1) * 512]
                            if (half * 4 + jc) % 4 == 3:
                                nc.scalar.copy(dst, pm[:])
                            else:
                                nc.vector.tensor_copy(out=dst, in_=pm[:])
                    # natural-layout row band: rows 64*ld4 .. 64*ld4+64 of
                    # each mix; one DMA per (half, h): 64 source partitions,
                    # one contiguous 128KB destination block
                    wengs = [nc.scalar, nc.gpsimd, nc.sync, nc.scalar]
                    for half in range(2):
                        for h in range(2):
                            base = 64 * ld4 + 32 * half + 16 * h
                            wengs[2 * half + h].dma_start(
                                out=nat_il[base : base + 16].rearrange(
                                    "x q j -> (x q) j"
                                ),
                                in_=stg[h * 64 : h * 64 + 64, half, :],
                            )

            # =========== Phases 2-4: three chained GEMMs (fp8 DR) ===========
            with (
                tc.tile_pool(name="big", bufs=1) as bigpool,
                tc.tile_pool(name="gw", bufs=3) as gpool,
                tc.tile_pool(name="nrm", bufs=4) as npool,
            ):
                mv = [
                    bigpool.tile([P, 16, N], fp8, tag="mv0", name="mva"),
                    bigpool.tile([P, 16, N], fp8, tag="mv1", name="mvb"),
                ]
                h2t_sb = bigpool.tile([P, 16, N], bf16, tag="h2t", name="h2t")

                # Build mv0 = a^T chunks by PE-transposing nat0 blocks (fp8)
                anat_v = nat[0].rearrange("(ib p) k -> p ib k", p=P)
                with tc.tile_pool(name="tps", bufs=2, space="PSUM") as tpsum:
                    for kc in range(16):
                        ld = gpool.tile([P, 16, P], fp8, tag="ld")
                        # strided 128B-segment read: split across both HWDGE
                        # rings for descriptor-rate headroom
                        nc.sync.dma_start(
                            out=ld[:, 0:8, :],
                            in_=anat_v[:, 0:8, kc * P : (kc + 1) * P],
                        )
                        nc.scalar.dma_start(
                            out=ld[:, 8:16, :],
                            in_=anat_v[:, 8:16, kc * P : (kc + 1) * P],
                        )
                        for ib4 in range(4):
                            # fp8 transpose writes PSUM at element step 2
                            tp = tpsum.tile(
                                [P, 512, 2], fp8, tag=f"tp{ib4 % 2}", name="tp"
                            )
                            for g in range(4):
                                nc.tensor.transpose(
                                    tp[:, g * P : (g + 1) * P, 0],
                                    ld[:, ib4 * 4 + g, :],
                                    ident8[:],
                                )
                            if ib4 % 2 == 0:
                                nc.vector.tensor_copy(
                                    out=mv[0][
                                        :, kc, ib4 * 512 : (ib4 + 1) * 512
                                    ],
                                    in_=tp[:, :, 0],
                                )
                            else:
                                nc.scalar.copy(
                                    mv[0][:, kc, ib4 * 512 : (ib4 + 1) * 512],
                                    tp[:, :, 0],
                                )

                def gemm(qi, rhs_res, out_res, gpsum):
                    """Transposed-chain GEMM: out = mix_q^T @ rhs, fp8 DR.

                    qi: q index in nat (1=b, 2=g1, 3=g2).
                    rhs_res: SBUF-resident moving operand [P, 16, N] fp8,
                        holding S * (previous normalized intermediate)^T.
                    out_res: SBUF [P, 16, N] fp8 -> normalize, evict with
                        dinv*S; None -> evict bf16 to h2t_sb with 1/(16*S).
                    """
                    natv = nat[qi].rearrange("(kc p) j -> p kc j", p=P)
                    for ms in range(16):
                        bts = gpool.tile([P, 16, P], fp8, tag="bts")
                        # split the strided stationary load across both rings
                        nc.sync.dma_start(
                            out=bts[:, 0:8, :],
                            in_=natv[:, 0:8, ms * P : (ms + 1) * P],
                        )
                        nc.scalar.dma_start(
                            out=bts[:, 8:16, :],
                            in_=natv[:, 8:16, ms * P : (ms + 1) * P],
                        )
                        ps = [
                            gpsum.tile(
                                [P, 512], f32, tag=f"ps{ic}", name=f"ps{ic}"
                            )
                            for ic in range(4)
                        ]
                        for kc2 in range(8):
                            for ic in range(4):
                                nc.tensor.matmul(
                                    ps[ic][:],
                                    lhsT=bts[:, 2 * kc2 : 2 * kc2 + 2, :],
                                    rhs=rhs_res[
                                        :,
                                        2 * kc2 : 2 * kc2 + 2,
                                        ic * 512 : (ic + 1) * 512,
                                    ],
                                    start=(kc2 == 0),
                                    stop=(kc2 == 7),
                                    perf_mode=DR,
                                )
                        if out_res is not None:
                            dc = (ms * P) // 512
                            v = ms % 4
                            degp = npool.tile([P, 4], f32, tag="degp")
                            # zero the diagonal in place + row-sum of masked
                            nc.vector.scalar_tensor_tensor(
                                out=ps[dc][:],
                                in0=ps[dc][:],
                                scalar=1.0,
                                in1=masks[:, v],
                                op0=MUL,
                                op1=MUL,
                                accum_out=degp[:, dc : dc + 1],
                            )
                            for ic in range(4):
                                if ic != dc:
                                    nc.vector.tensor_reduce(
                                        degp[:, ic : ic + 1], ps[ic][:], AX, ADD
                                    )
                            degs = npool.tile([P, 1], f32, tag="degs")
                            nc.vector.tensor_reduce(degs[:], degp[:], AX, ADD)
                            # dinv = S/deg, so the fp8 store sits near 0.5
                            degss = npool.tile([P, 1], f32, tag="degss")
                            nc.vector.tensor_scalar_mul(
                                degss[:], degs[:], 1.0 / S_SCALE
                            )
                            dinv = npool.tile([P, 1], f32, tag="dinv")
                            nc.vector.reciprocal(dinv[:], degss[:])
                            for ic in range(4):
                                nc.scalar.activation(
                                    out_res[:, ms, ic * 512 : (ic + 1) * 512],
                                    ps[ic][:],
                                    COPY,
                                    scale=dinv[:],
                                )
                        else:
                            for ic in range(4):
                                nc.scalar.activation(
                                    h2t_sb[:, ms, ic * 512 : (ic + 1) * 512],
                                    ps[ic][:],
                                    COPY,
                                    scale=1.0 / (16.0 * S_SCALE),
                                )

                with tc.tile_pool(name="gps", bufs=2, space="PSUM") as gpsum:
                    # GEMM1: Ht = b^T a^T ; normalize -> S*Hnt in mv[1]
                    gemm(1, mv[0], mv[1], gpsum)
                    # GEMM2: H't = g1^T (S Hnt) ; normalize -> S*H'nt
                    mv0b = bigpool.tile([P, 16, N], fp8, tag="mv0")
                    gemm(2, mv[1], mv0b, gpsum)
                    # GEMM3: H''t/16 = g2^T (S H'nt)/(16 S) -> h2t_sb (bf16)
                    gemm(3, mv0b, None, gpsum)

                # ===== Phase 5: local symmetrize G = (H'' + H''^T)/16 =====
                with tc.tile_pool(name="sps", bufs=2, space="PSUM") as spsum:
                    for ms in range(16):
                        tps = [
                            spsum.tile(
                                [P, 512], bf16, tag=f"sp{b4}", name="tps"
                            )
                            for b4 in range(4)
                        ]
                        for kc in range(16):
                            nc.tensor.transpose(
                                tps[kc // 4][
                                    :, (kc % 4) * P : (kc % 4 + 1) * P
                                ],
                                h2t_sb[:, kc, ms * P : (ms + 1) * P],
                                identb[:],
                            )
                        gst = gpool.tile([P, N], bf16, tag="gst")
                        for b4 in range(4):
                            eng = nc.vector
                            eng.scalar_tensor_tensor(
                                out=gst[:, b4 * 512 : (b4 + 1) * 512],
                                in0=h2t_sb[:, ms, b4 * 512 : (b4 + 1) * 512],
                                scalar=1.0,
                                in1=tps[b4][:],
                                op0=MUL,
                                op1=ADD,
                            )
                        weng = nc.scalar if ms % 2 else nc.sync
                        weng.dma_start(
                            out=gsym[ms * P : (ms + 1) * P, :], in_=gst[:]
                        )

                # ===== Phase 6: ReduceScatter -> this core's 256-row band ===
                nc.gpsimd.collective_compute(
                    "ReduceScatter",
                    ADD,
                    replica_groups=[list(range(NCORES))],
                    ins=[gsym[:].opt()],
                    outs=[g_sh[:].opt()],
                )

                # ===== Phase 7: copy the band out (d2d; host upcasts) =====
                nc.sync.dma_start(out=out_ext[:], in_=g_sh[:])

    nc.compile()
    return nc


def _get_program():
    global _PROGRAM
    if _PROGRAM is None:
        _PROGRAM = _build_program()
    return _PROGRAM


def _make_w4d(sws) -> np.ndarray:
    """Duplicated block-diagonal mix weights [128, 2, 128].

    w4d[(x*8+e), h, h2*64 + x*4 + q] = sws[q][e] if h==h2.
    With DoubleRow the two k-tiles (h) of the moving operand produce the
    two row blocks h2 of the packed (x, q) mix outputs; q is innermost so
    the mix eviction hits the row-interleaved nat layout contiguously.
    """
    w4d = np.zeros((P, 2, P), np.float32)
    for q, sw in enumerate(sws):
        for x in range(16):
            for h in range(2):
                w4d[x * 8 : (x + 1) * 8, h, h * 64 + x * 4 + q] = sw.astype(
                    np.float32
                )
    return w4d


def _prep_inputs(A, w1_0, w2_0, w_1, w_2):
    import ml_dtypes

    e4 = ml_dtypes.float8_e4m3
    swa = _softmax_rows(np.asarray(w1_0))
    swb = _softmax_rows(np.asarray(w2_0))
    sg1 = _softmax_rows(np.asarray(w_1))
    sg2 = _softmax_rows(np.asarray(w_2))

    a8 = np.asarray(A, dtype=np.float32)[0].astype(e4)  # [k, j, e]
    # At3[kb, (k16 e), j] = A[16*kb + k16, j, e]
    at3 = np.ascontiguousarray(a8.transpose(0, 2, 1).reshape(P, P, N))
    in_maps = []
    for c in range(NCORES):
        w4d = _make_w4d([swa[c], swb[c], sg1[c], sg2[c]]).astype(e4)
        in_maps.append({"At3": at3, "w4d": w4d})
    return in_maps


def kernel(A, w1_0, w2_0, w_1, w_2):
    from concourse.bass_utils import run_bass_kernel_spmd

    in_maps = _prep_inputs(A, w1_0, w2_0, w_1, w_2)
    nc = _get_program()
    res = run_bass_kernel_spmd(nc, in_maps, list(range(NCORES)))
    return np.concatenate(
        [
            np.asarray(res.results[k]["out"]).astype(np.float32)
            for k in range(NCORES)
        ],
        axis=0,
    )
